# revision 14
# baseline (speedup 1.0000x reference)
"""Trainium2 Bass kernel for nn_CoreferenceResolver (segment_reduce).

Reference computation (per batch b of 16):
  - gather 64 entity spans (4 tokens each) from x[b] (2048x768), max-pool -> emb [64,768]
  - pairwise cosine sim (64x64), standardized by (cos - thr) / (std+1e-5)
  - for all 2016 i<j pairs: feats=[sim, emb_i, emb_j] (1537) -> MLP 768/512/256/2

Sharding: data-parallel over batch, 2 batches per core on 8 cores.

Kernel restructuring (all matmul inputs bf16; PSUM accumulation f32):
  - layer-1 factorization: feats @ w1 = sim*w1[0] + emb_i @ w1A + emb_j @ w1B.
    EA = emb @ w1A, EB = emb @ w1B computed once per entity, then ONE
    selection matmul per (h-chunk, 512-pair-chunk) using the stacked
    stationary tile EAB_b = [EA rows i=0..62; EB rows j=1..63; w1row0; b1]
    (i=NE-1 never occurs as i, j=0 never as j, so 63+63+2 = 128 rows) against
    a constant 0/1 selection matrix SS whose row 126 carries the per-pair sim
    value (written at runtime) and row 127 is 1 (adds b1).
  - sim extraction per 512-pair chunk, batch-local: C = cosadj_b @ Sj_local,
    D = C .* Si_local, sim = ones^T @ D, scaled by 1/std at the copy into SS.
  - cosine sim via Gram matrix; per-batch std as in the reference
    (eps guards are folded/dropped: |err| ~1e-4, tolerance is 2e-2).
  - layer 4 computed transposed ([2, pairs]); final [pairs, 2] transpose is
    a free host-side reshape of the DRAM output.
  - PSUM evacuations (bias+relu) are split into 256-wide halves rotated
    across Act/DVE/Pool engines so PSUM banks recycle quickly.
Pairs padded 2016 -> 2048 per batch (pad columns have all-zero selectors).
"""

import numpy as np
import ml_dtypes

LAST_RESULT = None

import concourse.bass as bass
import concourse.mybir as mybir
import concourse.tile as tile
from concourse import bacc
from concourse.bass_utils import run_bass_kernel_spmd

F32 = mybir.dt.float32
BF = mybir.dt.bfloat16
I32 = mybir.dt.int32
BF16 = ml_dtypes.bfloat16

OP = mybir.AluOpType
ACT = mybir.ActivationFunctionType

B, L, H, NE, SPAN = 16, 2048, 768, 64, 4
EPS_COS = 1e-8
EPS_STD = 1e-5
N_CORES = 8
NB = B // N_CORES                 # batches per core = 2
NPAIR = NE * (NE - 1) // 2        # 2016
PADPAIR = 2048                    # padded pairs per batch
NPT = NB * PADPAIR                # 4096 padded pairs per core
NSLOT = NB * NE                   # 128 entity slots per core
HC = H // 128                     # 6 h-chunks
O2, O2C = 512, 4                  # layer2 out dim, chunks
O3, O3C = 256, 2                  # layer3 out dim, chunks
NPC = NPT // 512                  # 8 pair-chunks of 512
IU, JU = np.triu_indices(NE, k=1)


def _host_consts(w1, b1, w2, b2, w3, b3, w4, b4):
    """Constant tensors shared by all cores (host-precomputed)."""
    def bf(a):
        return np.ascontiguousarray(np.asarray(a, np.float32), dtype=BF16)

    c = {}
    c["wA"] = bf(w1[1:1 + H])            # [768,768]
    c["wB"] = bf(w1[1 + H:1 + 2 * H])    # [768,768]
    c["w2"] = bf(w2)                     # [768,512]
    c["w3"] = bf(w3)                     # [512,256]
    c["w4"] = bf(w4)                     # [256,2]
    c["w1b1"] = bf(np.stack([w1[0], b1]))        # [2,768]
    c["b2col"] = np.ascontiguousarray(b2.reshape(O2C, 128).T, np.float32)
    c["b3col"] = np.ascontiguousarray(b3.reshape(O3C, 128).T, np.float32)
    c["b4col"] = np.ascontiguousarray(b4.reshape(2, 1), np.float32)

    # stacked local selection matrix over padded pair columns.
    # Row layout (engine writes need partition base 0/32/64/96, so the
    # runtime-written sim row sits at 96):
    #   0..62  Si (i=0..62)         <- EA row i
    #   63..95 Sj (j=1..33)         <- EB row j     (row 62+j)
    #   96     sim (runtime)        <- w1row0
    #   97..126 Sj (j=34..63)       <- EB row j     (row 63+j)
    #   127    ones (real pairs)    <- b1
    SS = np.zeros((128, NPT), np.float32)
    jrow = np.where(JU <= 33, 62 + JU, 63 + JU)
    for bl in range(NB):
        cols = bl * PADPAIR + np.arange(NPAIR)
        SS[IU, cols] = 1.0
        SS[jrow, cols] = 1.0
        SS[127, cols] = 1.0
    c["SS"] = bf(SS)

    # per-batch Sj selector for the sim matmul: rows b*64+j, batch-local cols
    SjL = np.zeros((128, PADPAIR), np.float32)
    for bl in range(NB):
        SjL[bl * NE + JU, np.arange(NPAIR)] = 1.0
    c["SjL"] = bf(SjL)

    c["identB"] = bf(np.eye(128))
    c["ident"] = np.eye(128, dtype=np.float32)
    c["onesc"] = bf(np.ones((128, 1)))
    bm = np.zeros((128, 128), np.float32)
    for bl in range(NB):
        bm[bl * NE:(bl + 1) * NE, bl * NE:(bl + 1) * NE] = 1.0
    c["bmask"] = bm
    bsel = np.zeros((128, NB), np.float32)
    bsel[np.arange(128), np.arange(128) // NE] = 1.0
    c["bsel"] = bsel                                  # [128, 2]
    c["bselT"] = np.ascontiguousarray(bsel.T)         # [2, 128]
    return c


def _build_module(threshold: float, stage: int = 99):
    nc = bacc.Bacc("TRN2", target_bir_lowering=False, debug=False,
                   num_devices=N_CORES)

    din = {}

    def dram_in(name, shape, dtype=F32):
        din[name] = nc.dram_tensor(name, list(shape), dtype, kind="ExternalInput").ap()
        return din[name]

    xf = dram_in("xf", [NB * L, H])
    gidx = dram_in("gidx", [128, SPAN], I32)
    for name, *shape_extra in [
        ("wA", [H, H], BF), ("wB", [H, H], BF), ("w2", [H, O2], BF),
        ("w3", [O2, O3], BF), ("w4", [O3, 2], BF), ("w1b1", [2, H], BF),
        ("b2col", [128, O2C]), ("b3col", [128, O3C]), ("b4col", [2, 1]),
        ("SS", [128, NPT], BF), ("SjL", [128, PADPAIR], BF),
        ("identB", [128, 128], BF), ("ident", [128, 128]),
        ("onesc", [128, 1], BF),
        ("bmask", [128, 128]), ("bsel", [128, NB]), ("bselT", [NB, 128]),
    ]:
        dram_in(name, *shape_extra)
    out_d = nc.dram_tensor("out", [2, NPT], F32, kind="ExternalOutput").ap()

    with tile.TileContext(nc) as tc:
        with (
            tc.tile_pool(name="consts", bufs=1) as cp,
            tc.tile_pool(name="sp", bufs=2) as sp,
            tc.tile_pool(name="dp", bufs=2) as dpool,
            tc.tile_pool(name="otp", bufs=2) as otp,
            tc.tile_pool(name="h1p", bufs=2) as h1p,
            tc.tile_pool(name="h2p", bufs=2) as h2p,
            tc.tile_pool(name="h3p", bufs=2) as h3p,
            tc.tile_pool(name="ps0", bufs=1, space="PSUM") as ps0,
            tc.tile_pool(name="psL1", bufs=3, space="PSUM") as psL1,
            tc.tile_pool(name="psL2", bufs=2, space="PSUM") as psL2,
            tc.tile_pool(name="psL3", bufs=2, space="PSUM") as psL3,
        ):
            # ---- constant loads. ring A = sync(SP), ring B = scalar(Act) ----
            def load(name, shape, view=None, dtype=F32, ring=None):
                t = cp.tile(shape, dtype, tag=name, name=name)
                src = din[name] if view is None else view
                (ring or nc.sync).dma_start(out=t[:], in_=src)
                return t

            # ring A in dependency-criticality order
            gidx_sb = load("gidx", [128, SPAN], dtype=I32)
            ident_sb = load("ident", [128, 128])
            identB_sb = load("identB", [128, 128], dtype=BF)
            wA_sb = load("wA", [128, HC, H], din["wA"].rearrange("(c p) h -> p c h", p=128), dtype=BF)
            wB_sb = load("wB", [128, HC, H], din["wB"].rearrange("(c p) h -> p c h", p=128), dtype=BF)
            SS_sb = load("SS", [128, NPT], dtype=BF)
            SjL_sb = load("SjL", [128, PADPAIR], dtype=BF)
            w2_sb = load("w2", [128, HC, O2], din["w2"].rearrange("(c p) o -> p c o", p=128), dtype=BF)
            w3_sb = load("w3", [128, O2C, O3], din["w3"].rearrange("(c p) o -> p c o", p=128), dtype=BF)
            w4_sb = load("w4", [128, O3C, 2], din["w4"].rearrange("(c p) o -> p c o", p=128), dtype=BF)
            # ring B: small consts
            onesc_sb = load("onesc", [128, 1], dtype=BF, ring=nc.scalar)
            bmask_sb = load("bmask", [128, 128], ring=nc.scalar)
            bsel_sb = load("bsel", [128, NB], ring=nc.scalar)
            b2col_sb = load("b2col", [128, O2C], ring=nc.scalar)
            b3col_sb = load("b3col", [128, O3C], ring=nc.scalar)
            b4col_sb = load("b4col", [2, 1], ring=nc.scalar)
            bselT_sb = load("bselT", [NB, 128], ring=nc.scalar)
            # EAB stationary tiles; rows 126/127 (w1row0, b1) DMA'd from host
            EAB = [cp.tile([128, H], BF, tag=f"EAB{b}", name=f"EAB{b}")
                   for b in range(NB)]
            for b in range(NB):
                nc.scalar.dma_start(out=EAB[b][96:97, :], in_=din["w1b1"][0:1, :])
                nc.scalar.dma_start(out=EAB[b][127:128, :], in_=din["w1b1"][1:2, :])

            # ---- gather entity span rows + max-pool ----
            spans = cp.tile([128, SPAN, H], F32, tag="spans")
            for s in range(SPAN):
                nc.gpsimd.indirect_dma_start(
                    out=spans[:, s, :], out_offset=None,
                    in_=xf,
                    in_offset=bass.IndirectOffsetOnAxis(ap=gidx_sb[:, s:s + 1], axis=0),
                )
            emb = cp.tile([128, H], F32, tag="emb")
            tmpm = sp.tile([128, H], F32, tag="tmpm")
            tmpm2 = sp.tile([128, H], F32, tag="tmpm2")
            nc.vector.tensor_tensor(out=tmpm[:], in0=spans[:, 0, :], in1=spans[:, 1, :], op=OP.max)
            nc.vector.tensor_tensor(out=tmpm2[:], in0=spans[:, 2, :], in1=spans[:, 3, :], op=OP.max)
            nc.vector.tensor_tensor(out=emb[:], in0=tmpm[:], in1=tmpm2[:], op=OP.max)

            if stage >= 2:
                # ---- transpose emb -> embT chunks [h',6,e] (bf16) ----
                embT = cp.tile([128, HC, 128], BF, tag="embT")
                for hc in range(HC):
                    pt = psL1.tile([128, 128], F32, tag="psL1", name="pt")
                    nc.tensor.transpose(out=pt[:], in_=emb[:, hc * 128:(hc + 1) * 128],
                                        identity=ident_sb[:])
                    nc.vector.tensor_copy(out=embT[:, hc, :], in_=pt[:])

                # ---- Gram matrix first: its serial cosine chain (DVE/Act)
                #      overlaps the EA/EB matmuls on PE ----
                gram = psL1.tile([128, 128], F32, tag="psL1", name="gram")
                for hc in range(HC):
                    nc.tensor.matmul(out=gram[:], lhsT=embT[:, hc, :], rhs=embT[:, hc, :],
                                     start=(hc == 0), stop=(hc == HC - 1))

                # ---- EA = emb @ wA ; EB = emb @ wB  (no bias; b1 rides SS row 127) ----
                EA_sb = cp.tile([128, H], BF, tag="EA")
                EB_sb = cp.tile([128, H], BF, tag="EB")
                psE = {}
                for di in range(2):
                    for ni, (n0, nn_) in enumerate(((0, 512), (512, 256))):
                        pool = psL2 if di == 0 else psL3
                        psE[di, ni] = pool.tile([128, 512], F32, tag=pool.name,
                                                name=f"psE{di}{ni}")
                for di, w_sb in enumerate((wA_sb, wB_sb)):
                    for hc in range(HC):
                        for ni, (n0, nn_) in enumerate(((0, 512), (512, 256))):
                            nc.tensor.matmul(
                                out=psE[di, ni][:, :nn_], lhsT=embT[:, hc, :],
                                rhs=w_sb[:, hc, n0:n0 + nn_],
                                start=(hc == 0), stop=(hc == HC - 1))
                for di, dst in enumerate((EA_sb, EB_sb)):
                    for ni, (n0, nn_) in enumerate(((0, 512), (512, 256))):
                        if di == 0:
                            nc.vector.tensor_copy(out=dst[:, n0:n0 + nn_],
                                                  in_=psE[di, ni][:, :nn_])
                        else:
                            nc.scalar.copy(out=dst[:, n0:n0 + nn_],
                                           in_=psE[di, ni][:, :nn_])
                # scatter EA/EB rows into the stacked stationary tiles (SBUF->SBUF DMA)
                for b in range(NB):
                    nc.scalar.dma_start(out=EAB[b][0:63, :], in_=EA_sb[b * NE:b * NE + 63, :])
                    nc.scalar.dma_start(out=EAB[b][63:96, :], in_=EB_sb[b * NE + 1:b * NE + 34, :])
                    nc.scalar.dma_start(out=EAB[b][97:127, :], in_=EB_sb[b * NE + 34:b * NE + 64, :])

            if stage >= 3:
                # ---- cosine path (gram computed above) ----
                scratch = sp.tile([128, 128], F32, tag="scr128")
                dvec = sp.tile([128, 1], F32, tag="dvec")
                nc.vector.tensor_tensor(out=scratch[:], in0=gram[:], in1=identB_sb[:], op=OP.mult)
                nc.vector.tensor_reduce(out=dvec[:], in_=scratch[:],
                                        axis=mybir.AxisListType.X, op=OP.add)
                # inv = 1/sqrt(diag) (norms are O(sqrt(H)); eps guard dropped)
                inv = sp.tile([128, 1], F32, tag="inv")
                nc.vector.reciprocal(out=inv[:], in_=dvec[:])
                nc.scalar.activation(out=inv[:], in_=inv[:], func=ACT.Sqrt)
                g1 = sp.tile([128, 128], BF, tag="g1")
                nc.vector.tensor_scalar(out=g1[:], in0=gram[:], scalar1=inv[:, 0:1],
                                        scalar2=None, op0=OP.mult)
                g1t = psL1.tile([128, 128], BF, tag="psL1", name="g1t")
                nc.tensor.transpose(out=g1t[:], in_=g1[:], identity=identB_sb[:])
                cosm = sp.tile([128, 128], F32, tag="cosm")
                nc.vector.tensor_scalar(out=cosm[:], in0=g1t[:], scalar1=inv[:, 0:1],
                                        scalar2=None, op0=OP.mult)
                nc.vector.tensor_tensor(out=cosm[:], in0=cosm[:], in1=bmask_sb[:], op=OP.mult)

                # ---- per-batch std (ddof=1) over each 64x64 block ----
                rsbuf = sp.tile([128, 2], F32, tag="rsbuf")
                nc.vector.tensor_reduce(out=rsbuf[:, 0:1], in_=cosm[:],
                                        axis=mybir.AxisListType.X, op=OP.add)
                nc.vector.tensor_tensor(out=scratch[:], in0=cosm[:], in1=cosm[:], op=OP.mult)
                nc.vector.tensor_reduce(out=rsbuf[:, 1:2], in_=scratch[:],
                                        axis=mybir.AxisListType.X, op=OP.add)
                stats = psL1.tile([NB, 2], F32, tag="psL1", name="stats")
                nc.tensor.matmul(out=stats[:], lhsT=bsel_sb[:], rhs=rsbuf[:], start=True, stop=True)
                n_el = float(NE * NE)
                st = sp.tile([NB, 2], F32, tag="st")
                nc.vector.tensor_copy(out=st[:], in_=stats[:])
                var = sp.tile([NB, 1], F32, tag="var")
                nc.vector.tensor_tensor(out=var[:], in0=st[:, 0:1], in1=st[:, 0:1], op=OP.mult)
                nc.vector.tensor_scalar(out=var[:], in0=var[:], scalar1=-1.0 / n_el,
                                        scalar2=None, op0=OP.mult)
                nc.vector.tensor_tensor(out=var[:], in0=var[:], in1=st[:, 1:2], op=OP.add)
                nc.vector.tensor_scalar(out=var[:], in0=var[:], scalar1=1.0 / (n_el - 1.0),
                                        scalar2=None, op0=OP.mult)
                # rcpvar = 1/sqrt(var) ~= 1/(std+1e-5)
                rcpvar = sp.tile([NB, 1], F32, tag="rcpvar")
                nc.vector.reciprocal(out=rcpvar[:], in_=var[:])
                nc.scalar.activation(out=rcpvar[:], in_=rcpvar[:], func=ACT.Sqrt)
                rcpP_ps = psL1.tile([128, 1], F32, tag="psL1", name="rcpP_ps")
                nc.tensor.matmul(out=rcpP_ps[:], lhsT=bselT_sb[:], rhs=rcpvar[:],
                                 start=True, stop=True)
                rcpP = sp.tile([128, 1], F32, tag="rcpP")
                nc.vector.tensor_copy(out=rcpP[:], in_=rcpP_ps[:])
                cosadj = sp.tile([128, 128], BF, tag="cosadj")
                nc.vector.tensor_scalar(out=cosadj[:], in0=cosm[:], scalar1=float(threshold),
                                        scalar2=rcpP[:, 0:1], op0=OP.subtract, op1=OP.mult)

            if stage >= 4:
                # ---- sim rows: C = cosadj_b @ Sj_loc ; D = C .* Si_loc ;
                #      sim = 1^T D, scaled by rcpvar -> SS row 126.
                #      First chunks upfront, the rest interleaved into the
                #      main loop so the PE stream never waits on them. ----
                def sim_prep(c):
                    b = c // (NPC // NB)
                    cs = slice(c * 512, (c + 1) * 512)
                    cl = slice((c % (NPC // NB)) * 512, (c % (NPC // NB)) * 512 + 512)
                    psC = psL1.tile([128, 512], F32, tag="psL1", name="psC")
                    nc.tensor.matmul(out=psC[0:64, :],
                                     lhsT=cosadj[b * NE:b * NE + 64, b * NE:b * NE + 64],
                                     rhs=SjL_sb[b * NE:b * NE + 64, cl], start=True, stop=True)
                    D = dpool.tile([63, 512], BF, tag="D")
                    nc.vector.tensor_tensor(out=D[:], in0=psC[0:63, :],
                                            in1=SS_sb[0:63, cs], op=OP.mult)
                    psR = ps0.tile([1, 512], F32, tag="ps0", name="psR")
                    nc.tensor.matmul(out=psR[:], lhsT=onesc_sb[0:63, :], rhs=D[:],
                                     start=True, stop=True)
                    nc.scalar.copy(out=SS_sb[96:97, cs], in_=psR[:])

                nsim = 2 if stage >= 5 else NPC
                for c in range(nsim):
                    sim_prep(c)

            if stage >= 5:
                # ---- main MLP over pair-chunks of 512 ----
                ev = [0]   # evacuation engine rotator; halves on two engines

                def half_evac(dst, src, bias, relu):
                    # GPSIMD cannot access PSUM: rotate halves across Act/DVE
                    h = dst.shape[-1] // 2
                    for lo, hi in ((0, h), (h, 2 * h)):
                        e = ev[0] % 2
                        ev[0] += 1
                        d, s = dst[:, lo:hi], src[:, lo:hi]
                        if e == 0:
                            nc.scalar.activation(
                                out=d, in_=s, func=(ACT.Relu if relu else ACT.Identity),
                                bias=(bias if bias is not None else 0.0))
                        else:
                            if bias is None:
                                nc.vector.tensor_scalar(out=d, in0=s, scalar1=0.0,
                                                        scalar2=None, op0=OP.max)
                            elif relu:
                                nc.vector.tensor_scalar(out=d, in0=s, scalar1=bias,
                                                        scalar2=0.0, op0=OP.add, op1=OP.max)
                            else:
                                nc.vector.tensor_scalar(out=d, in0=s, scalar1=bias,
                                                        scalar2=None, op0=OP.add)

                def layer4(pc, h3T):
                    # layer 4, transposed: outT [2, pairs]. Emitted one chunk
                    # late so its h3T dependency never stalls the PE stream.
                    cs4 = slice(pc * 512, (pc + 1) * 512)
                    ps4 = ps0.tile([2, 512], F32, tag="ps0", name="ps4")
                    for kc in range(O3C):
                        nc.tensor.matmul(out=ps4[:], lhsT=w4_sb[:, kc, :], rhs=h3T[:, kc, :],
                                         start=(kc == 0), stop=(kc == O3C - 1))
                    outT = otp.tile([2, 512], F32, tag="outT")
                    nc.vector.tensor_scalar(out=outT[:], in0=ps4[:], scalar1=b4col_sb[:, 0:1],
                                            scalar2=None, op0=OP.add)
                    nc.sync.dma_start(out=out_d[:, cs4], in_=outT[:])

                prev = None
                for pc in range(NPC):
                    b = pc // (NPC // NB)
                    cs = slice(pc * 512, (pc + 1) * 512)

                    # layer 1: h1T [h, pairs] - one selection matmul per h-chunk
                    h1T = h1p.tile([128, HC, 512], BF, tag="h1T")
                    for hc in range(HC):
                        ps1 = psL1.tile([128, 512], F32, tag="psL1", name="ps1")
                        nc.tensor.matmul(out=ps1[:], lhsT=EAB[b][:, hc * 128:(hc + 1) * 128],
                                         rhs=SS_sb[:, cs], start=True, stop=True)
                        half_evac(h1T[:, hc, :], ps1[:], None, True)

                    # layer 2
                    h2T = h2p.tile([128, O2C, 512], BF, tag="h2T")
                    for oc in range(O2C):
                        ps2 = psL2.tile([128, 512], F32, tag="psL2", name="ps2")
                        for hc in range(HC):
                            nc.tensor.matmul(out=ps2[:], lhsT=w2_sb[:, hc, oc * 128:(oc + 1) * 128],
                                             rhs=h1T[:, hc, :], start=(hc == 0), stop=(hc == HC - 1))
                        half_evac(h2T[:, oc, :], ps2[:], b2col_sb[:, oc:oc + 1], True)

                    if pc + 2 < NPC:
                        sim_prep(pc + 2)

                    if prev is not None:
                        layer4(*prev)

                    # layer 3
                    h3T = h3p.tile([128, O3C, 512], BF, tag="h3T")
                    for oc in range(O3C):
                        ps3 = psL3.tile([128, 512], F32, tag="psL3", name="ps3")
                        for kc in range(O2C):
                            nc.tensor.matmul(out=ps3[:], lhsT=w3_sb[:, kc, oc * 128:(oc + 1) * 128],
                                             rhs=h2T[:, kc, :], start=(kc == 0), stop=(kc == O2C - 1))
                        half_evac(h3T[:, oc, :], ps3[:], b3col_sb[:, oc:oc + 1], True)
                    prev = (pc, h3T)

                layer4(*prev)

    nc.compile()
    return nc


def kernel(**inputs):
    import os
    stage = int(os.environ.get("KSTAGE", "99"))
    x = np.ascontiguousarray(np.asarray(inputs["x"]), dtype=np.float32)
    thr = float(np.asarray(inputs["threshold"]))
    es = np.asarray(inputs["entity_starts"]).astype(np.int64)
    w1 = np.asarray(inputs["w1"], np.float32)
    b1 = np.asarray(inputs["b1"], np.float32)
    w2 = np.asarray(inputs["w2"], np.float32)
    b2 = np.asarray(inputs["b2"], np.float32)
    w3 = np.asarray(inputs["w3"], np.float32)
    b3 = np.asarray(inputs["b3"], np.float32)
    w4 = np.asarray(inputs["w4"], np.float32)
    b4 = np.asarray(inputs["b4"], np.float32)

    consts = _host_consts(w1, b1, w2, b2, w3, b3, w4, b4)
    nc = _build_module(thr, stage)

    in_maps = []
    for c in range(N_CORES):
        xs = np.ascontiguousarray(x[NB * c:NB * (c + 1)].reshape(NB * L, H))
        gidx = np.empty((128, SPAN), np.int32)
        p = np.arange(128)
        base = (p // NE) * L + es[NB * c + p // NE, p % NE]
        for s in range(SPAN):
            gidx[:, s] = base + s
        in_maps.append({**consts, "xf": xs, "gidx": gidx})

    trace = bool(int(os.environ.get("KTRACE", "0")))
    res = run_bass_kernel_spmd(nc, in_maps, core_ids=list(range(N_CORES)),
                               trace=trace)
    global LAST_RESULT
    LAST_RESULT = res

    out = np.empty((B, NPAIR, 2), np.float32)
    for c in range(N_CORES):
        o = res.results[c]["out"]          # [2, NPT]
        for bl in range(NB):
            out[NB * c + bl] = o[:, bl * PADPAIR: bl * PADPAIR + NPAIR].T
    return out.reshape(B * NPAIR, 2)
